# revision 1
# baseline (speedup 1.0000x reference)
"""DiagonalwiseSeparableLayer on 8 Trainium2 cores.

Math: the masked dense conv is exactly a depthwise 3x3 conv (diagonal
entries of splitw), followed by a grouped 1x1 conv (8 groups, 32->64 ch).
Both stages are group-local, so they fuse into a single grouped 3x3 conv
with per-group weights  Wf[g, t, ci, co] = splitw[g, ty, tx, ci, ci] *
pw[ci, g*64 + co]  (t = ty*3 + tx), precomputed on host in fp64->bf16.

Sharding: data-parallel over batch, 2 images per core (16 / 8).

Per-core kernel (all engines overlapped via the Tile framework):
  1. in-DMA   : x rows, pixel-major [112 pix, 16 rows x 256 ch] fp32
  2. cast     : fp32 -> bf16 (DVE)
  3. in-T     : PE transpose [112 pix, 128 ch] -> psum bf16 [128 ch, 112]
  4. in-copy  : ACT psum -> x_chT SBUF, W-padded layout [ch, (H+2) x 128]
                (row stride 128; cols 0 and 113 zeroed, rows 0/H+1 zeroed)
  5. matmul   : grouped 3x3 conv as 9 accumulating tap-matmuls, 16-way
                32x32 tile_position packing (4 groups x 2 co-halves x
                2 pixel chunks concurrently = full PE array), bf16 in,
                fp32 psum out. N=512 pixels (4 padded rows) per chunk.
  6. stage1   : ACT compacting copy psum [co,512] -> sbuf [co, 448 valid]
  7. out-T    : PE transpose fp32 [co, 128 pix] -> psum [pix, co] (exact)
  8. stage2   : DVE copy psum -> sbuf staging [pix, 7 x 128 co]
  9. out-DMA  : [128 pix, 128 co] blocks, 512B-contiguous HBM runs
"""
import numpy as np
import ml_dtypes
from contextlib import ExitStack

import concourse.bacc as bacc
import concourse.tile as tile
from concourse import mybir, masks
from concourse.bass_utils import run_bass_kernel_spmd

N_CORES = 8
B, H, W, CIN, COUT = 16, 112, 112, 256, 512
G, CPG = 8, 32                  # groups, ch/group
BPC = B // N_CORES              # batch per core
RW = 128                        # padded row stride in x_chT
GUARD = 128                     # guard elements before/after data region
DATA0 = GUARD                   # data base offset in x_chT
PAIR_ROWS = 8                   # padded rows per checkerboard pair
ROWBLK = 16                     # image rows per input DMA block
DW_SPLIT = 4
BF16 = mybir.dt.bfloat16
F32 = mybir.dt.float32

_CACHE = {}


def _build(bpc=BPC, h=H, num_devices=N_CORES, sim_safe=False, reps=1,
           phases="full", inline_inputs=False, internal_out=False,
           mm_split=1):
    hp = h + 2
    xch_len = GUARD + hp * RW + GUARD
    npix = h * W
    npair = h // PAIR_ROWS

    nc = bacc.Bacc("TRN2", target_bir_lowering=False, debug=False,
                   num_devices=num_devices)
    if inline_inputs:
        rng = np.random.default_rng(7)
        x_d = nc.inline_tensor(
            rng.standard_normal((bpc, h, W, CIN)).astype(np.float32),
            name="x_const").ap()
        w_d = nc.inline_tensor(
            (rng.standard_normal((128, 2 * 9 * 128)) * 0.05
             ).astype(ml_dtypes.bfloat16), name="w_const").ap()
        wp_d = nc.inline_tensor(
            (rng.standard_normal((128, 2 * 2 * 128)) * 0.05
             ).astype(ml_dtypes.bfloat16), name="wp_const").ap()
    else:
        x_d = nc.dram_tensor("x", [bpc, h, W, CIN], F32,
                             kind="ExternalInput").ap()
        w_d = nc.dram_tensor("w", [128, 2 * 9 * 128], BF16,
                             kind="ExternalInput").ap()
        wp_d = nc.dram_tensor("wp", [128, 2 * 2 * 128], BF16,
                              kind="ExternalInput").ap()
    if internal_out:
        out_d = nc.dram_tensor("outscratch", [bpc * npix, COUT], F32).ap()
        nc.dram_tensor("out", [1, 1], F32, kind="ExternalOutput")
    else:
        out_d = nc.dram_tensor("out", [bpc * npix, COUT], F32,
                               kind="ExternalOutput").ap()

    with tile.TileContext(nc) as tc, ExitStack() as ctx:
        const = ctx.enter_context(tc.tile_pool(name="const", bufs=1))
        stg_pool = ctx.enter_context(tc.tile_pool(name="stg", bufs=2))
        stgbf_pool = ctx.enter_context(tc.tile_pool(name="stgbf", bufs=2))
        xch_pool = ctx.enter_context(tc.tile_pool(name="xch", bufs=3))
        sb2_pool = ctx.enter_context(tc.tile_pool(name="sb2", bufs=3))
        ps_in = ctx.enter_context(tc.tile_pool(name="psin", bufs=2, space="PSUM"))
        ps_dw = ctx.enter_context(tc.tile_pool(name="psdw", bufs=3, space="PSUM"))
        y_pool = ctx.enter_context(tc.tile_pool(name="ypool", bufs=3))
        ps_ot = ctx.enter_context(tc.tile_pool(name="psot", bufs=1, space="PSUM"))

        ident_bf = const.tile([128, 128], BF16)
        masks.make_identity(nc, ident_bf[:])
        ident_f32 = const.tile([128, 128], F32)
        masks.make_identity(nc, ident_f32[:])
        w_sb = const.tile([128, 2 * 9 * 128], BF16)
        nc.sync.dma_start(out=w_sb[:], in_=w_d[:])
        wp_sb = const.tile([128, 2 * 2 * 128], BF16)
        nc.sync.dma_start(out=wp_sb[:], in_=wp_d[:])

        for rep in range(reps):
          for b in range(bpc):
            # ---- input phase: load, cast, transpose into x_chT ----
            xch = [xch_pool.tile([128, xch_len], BF16, tag="xch",
                                 name=f"xch_{rep}_{b}_{i}")
                   for i in range(2)]
            for hh in range(2):
                t = xch[hh]
                nc.gpsimd.memset(t[:, 0:DATA0 + RW], 0.0)  # guard + row0
                nc.gpsimd.memset(t[:, DATA0 + (hp - 1) * RW:xch_len], 0.0)
                pads = t[:, DATA0 + RW:DATA0 + (hp - 1) * RW].rearrange(
                    "p (r w) -> p r w", w=RW)
                nc.gpsimd.memset(pads[:, :, 0:1], 0.0)      # col 0
                nc.gpsimd.memset(pads[:, :, 113:128], 0.0)  # cols 113-127

            for blk in range(h // ROWBLK):
                stg = stg_pool.tile([112, ROWBLK * CIN], F32, tag="stg")
                nc.sync.dma_start(
                    out=stg[:].rearrange("p (r c) -> p r c", r=ROWBLK),
                    in_=x_d[b, blk * ROWBLK:(blk + 1) * ROWBLK].transpose(
                        [1, 0, 2]))
                sbf = stgbf_pool.tile([112, ROWBLK * CIN], BF16, tag="stgbf")
                nc.vector.tensor_copy(sbf[:], stg[:])
                for sub in range(ROWBLK // 8):
                    for hh in range(2):
                        pt = ps_in.tile([128, 8 * 112], BF16, tag="psin")
                        for rr in range(8):
                            row = sub * 8 + rr
                            nc.tensor.transpose(
                                pt[:, rr * 112:(rr + 1) * 112],
                                sbf[0:112, row * CIN + hh * 128:
                                    row * CIN + hh * 128 + 128],
                                ident_bf[0:112, 0:112])
                        r0 = blk * ROWBLK + sub * 8 + 1  # first padded row
                        dst = xch[hh][:, DATA0 + r0 * RW:
                                      DATA0 + (r0 + 8) * RW].rearrange(
                            "p (r w) -> p r w", w=RW)[:, :, 1:113]
                        nc.scalar.copy(dst, pt[:].rearrange(
                            "p (r w) -> p r w", w=112))

            # ---- conv phase: depthwise 3x3 diag matmul, then pointwise ----
            for half in range(2 if phases != "in" else 0):
                xh = xch[half]
                for p in range(npair):
                    ys = []
                    for ck in range(2):
                        r0 = PAIR_ROWS * p + 4 * ck + 1
                        ydw = ps_dw.tile([128, 512], F32, tag="psdw",
                                         name=f"ydw_{rep}_{b}_{half}_{p}_{ck}")
                        if sim_safe:
                            nc.vector.memset(ydw[:], 0.0)
                        for t in range(9):
                            ty, tx = t // 3, t % 3
                            dlt = (ty - 1) * RW + (tx - 1)
                            wof = (half * 9 + t) * 128
                            for q in range(DW_SPLIT):
                                qq = 128 // DW_SPLIT
                                nc.tensor.matmul(
                                    ydw[q * qq:(q + 1) * qq, :],
                                    w_sb[q * qq:(q + 1) * qq,
                                         wof + q * qq:wof + (q + 1) * qq],
                                    xh[q * qq:(q + 1) * qq,
                                       DATA0 + r0 * RW + dlt:
                                       DATA0 + r0 * RW + dlt + 512],
                                    start=(t == 0 and not sim_safe),
                                    stop=(t == 8),
                                    skip_group_check=True,
                                    tile_position=(q * qq, q * qq),
                                )
                        y_sb = y_pool.tile([128, 512], BF16, tag="ysb",
                                           name=f"ysb_{rep}_{b}_{half}_{p}_{ck}")
                        nc.vector.tensor_copy(y_sb[:], ydw[:])
                        ys.append(y_sb)
                    # pointwise with y stationary: out[pix, co] direct
                    sb2 = sb2_pool.tile([128, PAIR_ROWS * 256], F32, tag="sb2")
                    for rr2 in range(PAIR_ROWS // 2):   # row pairs
                        po = ps_ot.tile([128, 512], F32, tag="psot",
                                        name=f"po_{rep}_{b}_{half}_{p}_{rr2}")
                        for r2 in range(2):
                            row = rr2 * 2 + r2          # row within pair
                            ysrc = ys[row // 4]
                            nc.tensor.matmul(
                                po[:, r2 * 256:(r2 + 1) * 256],
                                ysrc[:, (row % 4) * 128:
                                     (row % 4) * 128 + 128],
                                wp_sb[:, half * 256:half * 256 + 256],
                                start=True, stop=True,
                                skip_group_check=True,
                                tile_position=(0, 0),
                            )
                        nc.vector.tensor_copy(
                            sb2[:, rr2 * 512:(rr2 + 1) * 512], po[:])
                    if phases == "in+mm":
                        continue
                    # out-DMA: valid pixels (partitions 1..113), 1KB runs
                    pixbase = b * npix + p * PAIR_ROWS * 112
                    cobase = half * 256
                    dstd = out_d[pixbase:pixbase + PAIR_ROWS * 112,
                                 cobase:cobase + 256].rearrange(
                        "(r p) c -> p r c", p=112)
                    nc.sync.dma_start(
                        out=dstd,
                        in_=sb2[1:113, :].rearrange("p (r c) -> p r c", c=256))

    nc.compile()
    return nc


def _prep_weights(splitw, pw):
    """(depthwise diag, pointwise block-diag) weights, bf16."""
    sw = np.asarray(splitw, dtype=np.float64)
    pwf = np.asarray(pw, dtype=np.float64).reshape(CPG, COUT)
    diag = sw[:, :, :, np.arange(CPG), np.arange(CPG)]       # (G,3,3,ci)
    wdw = np.zeros((128, 2 * 9 * 128), dtype=np.float64)
    for g in range(G):
        half, g_loc = g // 4, g % 4
        for t in range(9):
            for ci in range(CPG):
                ch = 32 * g_loc + ci
                wdw[ch, (half * 9 + t) * 128 + ch] = diag[g, t // 3, t % 3, ci]
    wpw = np.zeros((128, 2 * 2 * 128), dtype=np.float64)
    for g in range(G):
        half, g_loc = g // 4, g % 4
        kk, mh = g_loc // 2, g_loc % 2
        wpw[32 * g_loc:32 * g_loc + 32,
            (half * 2 + kk) * 128 + mh * 64:
            (half * 2 + kk) * 128 + mh * 64 + 64] = \
            pwf[:, g * 64:(g + 1) * 64]
    return (wdw.astype(ml_dtypes.bfloat16), wpw.astype(ml_dtypes.bfloat16))


def kernel(x, splitw, pw):
    x = np.ascontiguousarray(np.asarray(x, dtype=np.float32))
    wdw_arr, wpw_arr = _prep_weights(splitw, pw)
    if "nc" not in _CACHE:
        _CACHE["nc"] = _build()
    nc = _CACHE["nc"]
    in_maps = [{"x": x[i * BPC:(i + 1) * BPC], "w": wdw_arr,
                "wp": wpw_arr}
               for i in range(N_CORES)]
    res = run_bass_kernel_spmd(nc, in_maps, list(range(N_CORES)))
    out = np.concatenate([res.results[i]["out"] for i in range(N_CORES)], axis=0)
    return out.reshape(B, H, W, COUT)



# revision 2
# speedup vs baseline: 1.0013x; 1.0013x over previous
"""DiagonalwiseSeparableLayer on 8 Trainium2 cores — v6.

Strategy vs baseline:
  - Host pre-transposes x to channel-major padded layout, bf16:
    [img, half(128ch), 114*114 grid] with zero borders (stride 114).
    Device input phase = pure DMA (no PE transposes, no DVE casts).
  - Output written channel-major bf16 [img, co, 12544 px]; host
    transposes back to NHWC fp32. Halves HBM write traffic.
  - DW 3x3 depthwise: 64x64 PE tile mode, 4 tiles = 2 chunks (456 px
    = 4 rows) concurrently; tap-outer loop over 2 chunk-pairs reuses
    each diag stationary for 4 matmuls. Odd chunks land partition-
    swapped in psum; consumed as-is by a row-swapped PW stationary.
  - PW 1x1 grouped conv: full-array matmul, stationary = block-diag
    pw weights [128 ci, 128 co] (2 per half x 2 parity), moving = y.
    Output [co, px] direct -> compacting psum->sbuf bf16 copy -> DMA.
"""
import numpy as np
import ml_dtypes
from contextlib import ExitStack

import concourse.bacc as bacc
import concourse.tile as tile
from concourse import mybir
from concourse.bass_utils import run_bass_kernel_spmd

N_CORES = 8
B, H, W, CIN, COUT = 16, 112, 112, 256, 512
G, CPG = 8, 32
BPC = B // N_CORES            # images per core
RS = 114                      # padded row stride
NPR = 114                     # padded rows
XLEN = RS * NPR               # 12996 cols per channel
GUARD = 64
CHUNK = 4 * RS                # 456 px (4 padded rows) per matmul chunk
NCH = H // 4                  # 28 chunks per half-image
SCN = NCH // 4                # 7 super-chunks (4 chunks each)
VPX = H * W                   # 12544 valid px per image
BF16 = mybir.dt.bfloat16
F32 = mybir.dt.float32

_CACHE = {}


def _build(num_devices=N_CORES):
    nc = bacc.Bacc("TRN2", target_bir_lowering=False, debug=False,
                   num_devices=num_devices)
    x_d = nc.dram_tensor("x", [BPC, 2, 128, XLEN], BF16,
                         kind="ExternalInput").ap()
    wdw_d = nc.dram_tensor("w", [128, 2 * 9 * 64], BF16,
                           kind="ExternalInput").ap()
    wpw_d = nc.dram_tensor("wp", [128, 2 * 2 * 2 * 128], BF16,
                           kind="ExternalInput").ap()
    out_d = nc.dram_tensor("out", [BPC, COUT, VPX], BF16,
                           kind="ExternalOutput").ap()

    xg = GUARD + XLEN + GUARD

    with tile.TileContext(nc) as tc, ExitStack() as ctx:
        const = ctx.enter_context(tc.tile_pool(name="const", bufs=1))
        xch_pool = ctx.enter_context(tc.tile_pool(name="xch", bufs=4))
        y_pool = ctx.enter_context(tc.tile_pool(name="ysb", bufs=2))
        stg_pool = ctx.enter_context(tc.tile_pool(name="stg", bufs=4))
        ps_dw = ctx.enter_context(tc.tile_pool(name="psdw", bufs=4,
                                               space="PSUM"))
        ps_pw = ctx.enter_context(tc.tile_pool(name="pspw", bufs=4,
                                               space="PSUM"))

        wdw_sb = const.tile([128, 2 * 9 * 64], BF16)
        nc.scalar.dma_start(out=wdw_sb[:], in_=wdw_d[:])
        wpw_sb = const.tile([128, 2 * 2 * 2 * 128], BF16)
        nc.scalar.dma_start(out=wpw_sb[:], in_=wpw_d[:])

        # ---- PE warmup: keep HAM busy while input DMA streams ----
        warm_w = const.tile([128, 512], BF16)
        nc.gpsimd.memset(warm_w[:], 0.0)
        warm_ps = ps_pw.tile([128, 512], F32, tag="pspw", name="warm")
        for i in range(24):
            nc.tensor.matmul(warm_ps[:], warm_w[:, 0:128], warm_w[:],
                             start=True, stop=True, skip_group_check=True,
                             tile_position=(0, 0))

        # ---- input DMA: first 2 half-images upfront on sync queue,
        # the rest staggered on the ACT queue mid-loop ----
        NPAIR = NCH // 2          # 14 pairs per half-image
        HIMGS = [(b, hh) for b in range(BPC) for hh in range(2)]
        xh = {}

        def load_x(b, hh, eng):
            t = xch_pool.tile([128, xg], BF16, tag="xch",
                              name=f"xch_{b}_{hh}")
            nc.gpsimd.memset(t[:, 0:GUARD], 0.0)
            nc.gpsimd.memset(t[:, GUARD + XLEN:xg], 0.0)
            qc = XLEN // 4
            for q in range(4):
                eng.dma_start(
                    out=t[:, GUARD + q * qc:GUARD + (q + 1) * qc],
                    in_=x_d[b, hh, :, q * qc:(q + 1) * qc])
            xh[(b, hh)] = t

        load_x(*HIMGS[0], nc.sync)
        load_x(*HIMGS[1], nc.scalar)

        # ---- compute: pipeline unit = chunk pair (2 chunks, 912 px) ----
        # PE stream software-pipelined: PW of pair k is emitted after DW
        # of pair k+1, so the PE never waits on the y psum->sbuf copies.
        def dw_pair(b, hh, pr):
            xt = xh[(b, hh)]
            pdw = [ps_dw.tile([128, CHUNK], F32, tag="psdw",
                              name=f"dw_{b}_{hh}_{pr}_{c}")
                   for c in range(2)]
            ce = 2 * pr
            for t in range(9):
                ty, tx = t // 3 - 1, t % 3 - 1
                dt = ty * RS + tx
                for ch2 in range(2):             # channel 64-half
                    wsl = wdw_sb[64 * ch2:64 * ch2 + 64,
                                 (hh * 9 + t) * 64:
                                 (hh * 9 + t) * 64 + 64]
                    for par in range(2):         # chunk parity
                        c = ce + par
                        w0 = GUARD + (4 * c + 1) * RS + dt
                        jj = ch2 if par == 0 else 1 - ch2
                        nc.tensor.matmul(
                            pdw[par][64 * jj:64 * jj + 64, :],
                            wsl,
                            xt[64 * ch2:64 * ch2 + 64, w0:w0 + CHUNK],
                            start=(t == 0), stop=(t == 8),
                            skip_group_check=True,
                            tile_position=(64 * ch2, 64 * jj),
                        )
            # y copies (DVE/ACT queues; PE does not wait here)
            ysb = y_pool.tile([128, 2 * CHUNK], BF16, tag="ysb",
                              name=f"ysb_{b}_{hh}_{pr}")
            nc.vector.tensor_copy(ysb[:, 0:CHUNK], pdw[0][:])
            nc.scalar.copy(ysb[:, CHUNK:2 * CHUNK], pdw[1][:])
            return ysb

        # staging covers 2 pairs (one super-chunk) for 3584B DMA runs
        stg_cur = {}

        def pw_pair(b, hh, pr, ysb):
            if pr % 2 == 0:
                stg_cur[0] = [stg_pool.tile([128, 4 * 448], BF16,
                                            tag="stg",
                                            name=f"stg_{b}_{hh}_{pr}_{c}")
                              for c in range(2)]
            stg = stg_cur[0]
            po = (pr % 2) * 2          # pair offset in staging (chunks)
            for par in range(2):
                for co2 in range(2):
                    ppw = ps_pw.tile([128, CHUNK], F32, tag="pspw",
                                     name=f"pw_{b}_{hh}_{pr}_{par}_{co2}")
                    nc.tensor.matmul(
                        ppw[:],
                        wpw_sb[:, ((par * 2 + hh) * 2 + co2) * 128:
                               ((par * 2 + hh) * 2 + co2) * 128 + 128],
                        ysb[:, par * CHUNK:(par + 1) * CHUNK],
                        start=True, stop=True,
                        skip_group_check=True,
                        tile_position=(0, 0),
                    )
                    psrc = ppw[:].rearrange("p (r w) -> p r w", w=RS)
                    pdst = stg[co2][:, (po + par) * 448:
                                    (po + par + 1) * 448
                                    ].rearrange("p (r w) -> p r w", w=112)
                    if (par + co2) % 2 == 0:
                        nc.vector.tensor_copy(pdst, psrc[:, :, 1:113])
                    else:
                        nc.scalar.copy(pdst, psrc[:, :, 1:113])
            last_sc = (b, hh) == HIMGS[-1] and pr >= NPAIR - 2
            if last_sc:
                # tail trim: flush per pair so the final DMAs start early
                sc = pr // 2
                for co2 in range(2):
                    nc.sync.dma_start(
                        out=out_d[b, hh * 256 + co2 * 128:
                                  hh * 256 + co2 * 128 + 128,
                                  sc * 1792 + po * 448:
                                  sc * 1792 + (po + 2) * 448],
                        in_=stg[co2][:, po * 448:(po + 2) * 448])
            elif pr % 2 == 1:
                sc = pr // 2
                for co2 in range(2):
                    nc.sync.dma_start(
                        out=out_d[b, hh * 256 + co2 * 128:
                                  hh * 256 + co2 * 128 + 128,
                                  sc * 1792:(sc + 1) * 1792],
                        in_=stg[co2][:])

        work = [(b, hh, pr) for (b, hh) in HIMGS for pr in range(NPAIR)]
        prev = None
        for wi, (b, hh, pr) in enumerate(work):
            if pr == 5 and (b, hh) != HIMGS[-1]:
                nxt = HIMGS[HIMGS.index((b, hh)) + 1]
                if nxt not in xh:
                    load_x(*nxt, nc.scalar)
            ysb = dw_pair(b, hh, pr)
            if prev is not None:
                pw_pair(*prev)
            prev = (b, hh, pr, ysb)
        pw_pair(*prev)

    nc.compile()
    return nc


def _prep_x(x):
    """(B,H,W,256) fp32 -> (B, 2, 128, 114*114) bf16 padded ch-major."""
    xb = np.asarray(x, dtype=np.float32).astype(ml_dtypes.bfloat16)
    xp = np.zeros((B, 2, 128, NPR, RS), dtype=ml_dtypes.bfloat16)
    xp[:, :, :, 1:113, 1:113] = xb.transpose(0, 3, 1, 2).reshape(
        B, 2, 128, H, W)
    return np.ascontiguousarray(xp.reshape(B, 2, 128, XLEN))


def _prep_weights(splitw, pw):
    sw = np.asarray(splitw, dtype=np.float64)
    pwf = np.asarray(pw, dtype=np.float64).reshape(CPG, COUT)
    diag = sw[:, :, :, np.arange(CPG), np.arange(CPG)]   # (G,3,3,ci)
    # DW: [128 rows, (hh, tap) x 64] diag within each 64-block
    wdw = np.zeros((128, 2 * 9 * 64), dtype=np.float64)
    for p in range(128):
        for hh in range(2):
            ch = hh * 128 + p
            g, ci = ch // 32, ch % 32
            for t in range(9):
                wdw[p, (hh * 9 + t) * 64 + (p % 64)] = \
                    diag[g, t // 3, t % 3, ci]
    # PW: [128 rows, (parity, hh, co2) x 128]
    wpw = np.zeros((128, 2 * 2 * 2 * 128), dtype=np.float64)
    for par in range(2):
        for hh in range(2):
            for co2 in range(2):
                blk = ((par * 2 + hh) * 2 + co2) * 128
                for p in range(128):
                    prow = p if par == 0 else (p + 64) % 128
                    ci_g = hh * 128 + prow
                    for m in range(128):
                        co_g = hh * 256 + co2 * 128 + m
                        if ci_g // 32 == co_g // 64:
                            wpw[p, blk + m] = pwf[ci_g % 32, co_g]
    return (wdw.astype(ml_dtypes.bfloat16), wpw.astype(ml_dtypes.bfloat16))


def _post_out(res_list):
    """8 x (BPC, 512, 12544) bf16 -> (B,H,W,512) fp32."""
    o = np.concatenate(res_list, axis=0)           # (B, 512, 12544)
    return np.ascontiguousarray(
        o.astype(np.float32).transpose(0, 2, 1)).reshape(B, H, W, COUT)


def _in_maps(inputs):
    xt = _prep_x(inputs["x"])
    wdw_arr, wpw_arr = _prep_weights(inputs["splitw"], inputs["pw"])
    return [{"x": xt[i * BPC:(i + 1) * BPC], "w": wdw_arr,
             "wp": wpw_arr}
            for i in range(N_CORES)]


def kernel(x, splitw, pw):
    in_maps = _in_maps({"x": x, "splitw": splitw, "pw": pw})
    if "nc" not in _CACHE:
        _CACHE["nc"] = _build()
    nc = _CACHE["nc"]
    res = run_bass_kernel_spmd(nc, in_maps, list(range(N_CORES)))
    return _post_out([res.results[i]["out"] for i in range(N_CORES)])
